# revision 5
# baseline (speedup 1.0000x reference)
"""Trainium2 Bass kernel for nn_CachedMLP (2-expert dense MoE MLP).

Computation (reference):
    ew = expert_weights, swapped if expert_ids[0] != 0
    for e in {0,1}:  down_e = (silu(x @ w1_e.T) * (x @ w3_e.T)) @ w2_e
    out = down_0 * ew[0] + down_1 * ew[1]

Sharding: expert-parallel x tensor-parallel. Core c handles expert c//4
and rows [r*2867, (r+1)*2867) of that expert's w1/w3/w2 (r = c%4),
zero-padded to 2944 = 23*128. ew[e] (and the fp8 weight scale) is
folded into w2 on the host, so the sum of the 8 per-core partial
outputs is the final result.

Precision: w1/w3 are stored as fp8-E3M4 (scaled x64 on host; the PE
upconverts exactly, and silu de-scales via activation's scale=1/64);
w2, x, h stay bf16. Measured end-to-end rel err ~1.6% (tolerance 2e-2).
This halves the w13 HBM traffic: per-core reads drop 71.5 -> ~48 MB,
and the kernel is HBM-wire bound.

Device kernel per core:
  pass 1 over 6 a-slices (512 wide, last 384): for each d-chunk kd,
      gate[t, a-slice] += xT(kd) .T-matmul w1_tile(kd)   (PSUM, N=512)
      up[t, a-slice]   += xT(kd) .T-matmul w3_tile(kd)
    (x is the stationary operand -- loaded once per kd, amortized over
    both matmuls; the moving operand is the fp8 weight tile at N=512,
    so the PE streams near peak instead of reloading weights per MM).
    Then silu(scale=1/64)*up -> h[t, a-slice] (bf16), and PE-transpose
    128-col chunks into hT[a, t] for pass 2.
  pass 2: down[t, d-quarter] += hT(ka).T-matmul w2_tile(ka, q),
    accumulated over all 23 ka in 2 PSUM banks per 1024-wide quarter,
    then copied to SBUF and DMA'd out as bf16 partials.

DMA rings: w13 on SP (sync), w2 on GPSIMD so ~8 w2 tiles prefetch
during pass 1 without head-of-line blocking behind w13, x/ident/out
on ACT (scalar).
"""

import json
import os

import ml_dtypes
import numpy as np

T = 128          # tokens
D = 4096         # hidden dim
ACTIVE = 11468   # sparsity-selected neurons per expert
NCORES = 8
ASH = ACTIVE // 4        # 2867 active rows per core
NKA = 23                 # 128-row a-chunks per core
APAD = NKA * 128         # 2944
NKD = D // 128           # 32 d-chunks
JW_LAST = ASH - (NKA - 1) * 128  # 51 useful rows in the last a-chunk
NSL = 6                  # a-slices in pass 1
SLW = 512                # slice width (last slice: 384)
SLW_LAST = APAD - 5 * SLW  # 384
NQ = 4                   # pass-2 d-quarters
QW = D // NQ             # 1024

WSCALE = 64.0            # fp8 weight scale for w1/w3

BF16 = ml_dtypes.bfloat16
E3M4 = ml_dtypes.float8_e3m4

_EVENTSEM_CAP = 2


def _split_multi_waits(bir_json: bytes) -> bytes:
    """Hoist excess per-instruction sync waits into standalone waits.

    The axon-path walrus build accepts at most 1 sync-wait command per
    instruction (2 for EventSemaphore); Tile's wait assigner can emit
    more. Extra waits become wait-only EventSemaphore instructions
    inserted just before the offender on the same engine stream, which
    preserves semantics (the engine would have blocked there anyway).
    """
    d = json.loads(bir_json)
    for func in d.get("functions", []):
        for blk in func.get("blocks", []):
            out = []
            for inst in blk.get("instructions", []):
                sync = inst.get("sync_info")
                waits = (sync or {}).get("on_wait") or []
                cap = _EVENTSEM_CAP if inst.get("opcode") == "EventSemaphore" else 1
                if len(waits) > cap:
                    extra, keep = waits[:-cap], waits[-cap:]
                    for j in range(0, len(extra), _EVENTSEM_CAP):
                        w_inst = {
                            "engine": inst["engine"],
                            "ins": [],
                            "name": f"{inst['name']}-hw{j}",
                            "opcode": "EventSemaphore",
                            "outs": [],
                            "sync_info": {
                                "on_update": [],
                                "on_wait": extra[j : j + _EVENTSEM_CAP],
                            },
                        }
                        if "debug" in inst:
                            w_inst["debug"] = inst["debug"]
                        out.append(w_inst)
                    sync["on_wait"] = keep
                out.append(inst)
            blk["instructions"] = out
    return json.dumps(d).encode()


def _hoist_head_dmas(bir_json: bytes, max_hoist: int = 3) -> bytes:
    """Move the leading wait-free SP DMACopies into the preamble block.

    Tile's prologue (engine reg-init + const memsets + all-engine
    barrier) takes ~7us before the first dma_start issues, leaving the
    HBM wire idle. The first input DMAs have no waits and their
    destination tiles are disjoint from everything the preamble writes
    (asserted below), so issuing them before the barrier is safe: HWDGE
    keeps per-engine FIFO order and their sem increments are only ever
    waited on with >= thresholds.
    """
    d = json.loads(bir_json)
    for func in d.get("functions", []):
        blocks = func.get("blocks", [])
        if len(blocks) < 2:
            continue
        main, tile_blk = blocks[0], blocks[1]
        if main.get("name") != "main" or not tile_blk.get("name", "").startswith(
            "tile_context"
        ):
            continue
        # preamble must write only const tiles, else hoisting is unsafe
        pre_outs = {
            o.get("memref")
            for inst in main["instructions"]
            for o in inst.get("outs", [])
            if isinstance(o, dict)
        }
        if any(m and not m.startswith("const-") for m in pre_outs):
            continue
        hoisted = []
        remaining = []
        for inst in tile_blk["instructions"]:
            if (
                len(hoisted) < max_hoist
                and inst.get("engine") == "SP"
                and inst.get("opcode") == "DMACopy"
                and not ((inst.get("sync_info") or {}).get("on_wait"))
            ):
                hoisted.append(inst)
            else:
                remaining.append(inst)
        if not hoisted:
            continue
        ins_at = next(
            (
                i
                for i, inst in enumerate(main["instructions"])
                if inst.get("engine") == "SP" and inst.get("opcode") == "Drain"
            ),
            None,
        )
        if ins_at is None:
            continue
        main["instructions"][ins_at:ins_at] = hoisted
        tile_blk["instructions"] = remaining
    return json.dumps(d).encode()


def _install_wait_split():
    import concourse.bass2jax as b2j
    import concourse.bass_utils as bu

    if getattr(bu.compile_bir_kernel, "_wait_split", False):
        return
    orig = bu.compile_bir_kernel

    def compile_with_split(bir_json, tmpdir, neff_name="file.neff"):
        return orig(_split_multi_waits(_hoist_head_dmas(bir_json)), tmpdir, neff_name)

    compile_with_split._wait_split = True
    bu.compile_bir_kernel = compile_with_split
    if getattr(b2j, "compile_bir_kernel", None) is orig:
        b2j.compile_bir_kernel = compile_with_split


_program = None


def _build_program():
    """Build the single-core Bass/Tile program (same program on all 8 cores)."""
    import concourse.bass as bass
    import concourse.mybir as mybir
    from concourse.tile import TileContext

    f32 = mybir.dt.float32
    bf16 = mybir.dt.bfloat16
    fp8e3 = mybir.dt.float8e3
    Silu = mybir.ActivationFunctionType.Silu

    nc = bass.Bass()
    xb = nc.declare_dram_parameter("xb", [128, D], bf16, isOutput=False)
    ident = nc.declare_dram_parameter("ident", [128, 128], bf16, isOutput=False)
    w13a = nc.declare_dram_parameter(
        "w13a", [NSL - 1, NKD, 128, 2 * SLW], fp8e3, isOutput=False
    )
    w13b = nc.declare_dram_parameter(
        "w13b", [NKD, 128, 2 * SLW_LAST], fp8e3, isOutput=False
    )
    w2q = nc.declare_dram_parameter("w2q", [NQ, NKA, 128, QW], bf16, isOutput=False)
    out = nc.declare_dram_parameter("out", [T, D], bf16, isOutput=True)

    def slw_of(s):
        return SLW_LAST if s == NSL - 1 else SLW

    def jw_of(ka):
        return JW_LAST if ka == NKA - 1 else 128

    with TileContext(nc) as tc:
        with (
            tc.tile_pool(name="singles", bufs=1) as singles,
            tc.tile_pool(name="w13p", bufs=6) as w13p,
            tc.tile_pool(name="w2p", bufs=8) as w2p,
            tc.tile_pool(name="gap", bufs=2) as gap,
            tc.tile_pool(name="hp", bufs=2) as hp,
            tc.tile_pool(name="outp", bufs=2) as outp,
            tc.tile_pool(name="psum_ug", bufs=2, space="PSUM") as psum_ug,
            tc.tile_pool(name="psum_t", bufs=1, space="PSUM") as psum_t,
            tc.tile_pool(name="psum_o", bufs=1, space="PSUM") as psum_o,
        ):
            # x + ident on the ACT HWDGE ring so they overlap w13[0] on SP's
            xb_s = singles.tile([128, D], bf16)
            nc.scalar.dma_start(out=xb_s, in_=xb[:, :])
            id_s = singles.tile([128, 128], bf16)
            nc.scalar.dma_start(out=id_s, in_=ident[:, :])
            hT = singles.tile([128, APAD], bf16)

            # pass 1: gate/up in [t, a] layout, x stationary, fp8 w13 moving
            for s in range(NSL):
                sw = slw_of(s)
                gate_ps = psum_ug.tile([128, sw], f32, name=f"g{s}", tag="gate")
                up_ps = psum_ug.tile([128, sw], f32, name=f"u{s}", tag="up")
                for kd in range(NKD):
                    w13t = w13p.tile([128, 2 * sw], fp8e3)
                    if s < NSL - 1:
                        nc.sync.dma_start(out=w13t, in_=w13a[s, kd, :, :])
                    else:
                        nc.sync.dma_start(out=w13t, in_=w13b[kd, :, :])
                    lhsT = xb_s[:, kd * 128 : (kd + 1) * 128]
                    nc.tensor.matmul(
                        gate_ps,
                        lhsT,
                        w13t[:, :sw],
                        start=(kd == 0),
                        stop=(kd == NKD - 1),
                    )
                    nc.tensor.matmul(
                        up_ps,
                        lhsT,
                        w13t[:, sw : 2 * sw],
                        start=(kd == 0),
                        stop=(kd == NKD - 1),
                    )
                ga = gap.tile([128, sw], f32)
                nc.scalar.activation(
                    out=ga, in_=gate_ps, func=Silu, scale=1.0 / WSCALE
                )
                hsl = hp.tile([128, sw], bf16)
                nc.vector.tensor_mul(out=hsl, in0=ga, in1=up_ps)
                for c in range(sw // 128):
                    ka = s * 4 + c
                    tp = psum_t.tile([128, 128], bf16, name=f"t{ka}", tag="tp")
                    nc.tensor.transpose(
                        tp, hsl[:, c * 128 : (c + 1) * 128], id_s
                    )
                    nc.vector.tensor_copy(
                        out=hT[:, ka * 128 : (ka + 1) * 128], in_=tp
                    )

            # pass 2: down projection in 4 d-quarters; each quarter
            # accumulates across all ka in 2 PSUM banks, then its output
            # copy + DMA overlaps the next quarter's compute.
            for q in range(NQ):
                ops = psum_o.tile([128, QW], f32, name=f"o{q}", tag="od")
                for ka in range(NKA):
                    jw = jw_of(ka)
                    w2t = w2p.tile([128, QW], bf16)
                    nc.gpsimd.dma_start(out=w2t[:jw], in_=w2q[q, ka, :jw, :])
                    lhsT = hT[:jw, ka * 128 : (ka + 1) * 128]
                    for b in range(2):
                        nc.tensor.matmul(
                            ops[:, b * 512 : (b + 1) * 512],
                            lhsT,
                            w2t[:jw, b * 512 : (b + 1) * 512],
                            start=(ka == 0),
                            stop=(ka == NKA - 1),
                        )
                oth = outp.tile([T, QW], bf16, name=f"oth{q}", tag="oth")
                nc.vector.tensor_copy(out=oth, in_=ops)
                # output store on the ACT ring: doesn't queue behind the
                # remaining w2 transfers on the DVE FIFO ring
                nc.scalar.dma_start(out=out[:, q * QW : (q + 1) * QW], in_=oth)

    return nc


def _pack_w13(w1s: np.ndarray, w3s: np.ndarray):
    """[ASH, D] f32 pair -> ([NSL-1, NKD, 128, 2*SLW], [NKD, 128, 2*SLW_LAST])
    e3m4 blobs, scaled by WSCALE.

    blob[s, kd, p, j] = w1s[s*SLW + j, kd*128 + p] for j < sw,
                        w3s[s*SLW + (j-sw), kd*128 + p] for j >= sw.
    """
    padded = np.zeros((2, NSL * SLW, D), dtype=E3M4)
    padded[0, :ASH] = (w1s * np.float32(WSCALE)).astype(E3M4)
    padded[1, :ASH] = (w3s * np.float32(WSCALE)).astype(E3M4)
    # [2, NSL, SLW, NKD, 128] -> [NSL, NKD, 128, 2, SLW]
    r = padded.reshape(2, NSL, SLW, NKD, 128).transpose(1, 3, 4, 0, 2)
    a = np.ascontiguousarray(r[: NSL - 1]).reshape(NSL - 1, NKD, 128, 2 * SLW)
    b = np.ascontiguousarray(r[NSL - 1, :, :, :, :SLW_LAST]).reshape(
        NKD, 128, 2 * SLW_LAST
    )
    return a, b


def _pack_w2(w2s: np.ndarray, scale: float) -> np.ndarray:
    """[ASH, D] f32 -> [NQ, NKA, 128, QW] bf16 blob, scale folded in."""
    p = np.zeros((APAD, D), dtype=BF16)
    p[:ASH] = (w2s * np.float32(scale)).astype(BF16)
    return np.ascontiguousarray(p.reshape(NKA, 128, NQ, QW).transpose(2, 0, 1, 3))


def _pack_x(x: np.ndarray) -> np.ndarray:
    """[T, D] f32 -> [128, D] bf16: xb[p, kd*128 + t] = x[t, kd*128 + p]."""
    return (
        x.astype(BF16).reshape(T, NKD, 128).transpose(2, 1, 0).reshape(128, NKD * T)
    )


def make_in_maps(
    hidden_states,
    expert_weights,
    expert_ids,
    w1_e0,
    w3_e0,
    w2_e0,
    w1_e1,
    w3_e1,
    w2_e1,
):
    ids = np.asarray(expert_ids).reshape(-1)
    ew = np.asarray(expert_weights, dtype=np.float32).reshape(-1)
    if int(ids[0]) != 0:
        ew = ew[::-1]

    xb = _pack_x(np.asarray(hidden_states, dtype=np.float32))
    ident = np.eye(128, dtype=BF16)
    w1 = (np.asarray(w1_e0, np.float32), np.asarray(w1_e1, np.float32))
    w3 = (np.asarray(w3_e0, np.float32), np.asarray(w3_e1, np.float32))
    w2 = (np.asarray(w2_e0, np.float32), np.asarray(w2_e1, np.float32))

    in_maps = []
    for core in range(NCORES):
        e, r = divmod(core, 4)
        rows = slice(r * ASH, (r + 1) * ASH)
        w13a, w13b = _pack_w13(w1[e][rows], w3[e][rows])
        # h is scaled by WSCALE (up de-scale folded here), so w2 gets
        # ew / WSCALE.
        w2blob = _pack_w2(w2[e][rows], float(ew[e]) / WSCALE)
        in_maps.append(
            {
                "xb": xb,
                "ident": ident,
                "w13a": w13a,
                "w13b": w13b,
                "w2q": w2blob,
            }
        )
    return in_maps


LAST_RESULT = None


def kernel(**inputs) -> np.ndarray:
    global _program, LAST_RESULT
    _install_wait_split()
    from concourse.bass_utils import run_bass_kernel_spmd

    if _program is None:
        _program = _build_program()
        # Apply the BIR transforms at serialization time so the embedded
        # ant_bir payload (the compile-cache key) reflects them. Both
        # transforms are idempotent, so compile_bir_kernel re-applying
        # them is harmless.
        orig_tjb = _program.to_json_bytes

        def _tjb():
            return _split_multi_waits(_hoist_head_dmas(orig_tjb()))

        _program.to_json_bytes = _tjb

    in_maps = make_in_maps(**inputs)
    res = run_bass_kernel_spmd(
        _program,
        in_maps,
        core_ids=list(range(NCORES)),
        trace=bool(int(os.environ.get("KERNEL_TRACE", "0"))),
    )
    LAST_RESULT = res
    out = np.zeros((T, D), dtype=np.float32)
    for r in res.results:
        out += np.asarray(r["out"]).astype(np.float32)
    return out


# revision 15
# speedup vs baseline: 1.0071x; 1.0071x over previous
"""Trainium2 Bass kernel for nn_CachedMLP (2-expert dense MoE MLP).

Computation (reference):
    ew = expert_weights, swapped if expert_ids[0] != 0
    for e in {0,1}:  down_e = (silu(x @ w1_e.T) * (x @ w3_e.T)) @ w2_e
    out = down_0 * ew[0] + down_1 * ew[1]

Sharding: expert-parallel x tensor-parallel. Core c handles expert c//4
and rows [r*2867, (r+1)*2867) of that expert's w1/w3/w2 (r = c%4),
zero-padded to 2944 = 23*128. ew[e] (and the fp8 weight scale) is
folded into w2 on the host, so the sum of the 8 per-core partial
outputs is the final result.

Precision: w1/w3 are stored as fp8-E3M4 (scaled x64 on host; the PE
upconverts exactly, and silu de-scales via activation's scale=1/64);
w2, x, h stay bf16. Measured end-to-end rel err ~1.6% (tolerance 2e-2).
This halves the w13 HBM traffic: per-core reads drop 71.5 -> ~48 MB,
and the kernel is HBM-wire bound.

Device kernel per core:
  pass 1 over 6 a-slices (512 wide, last 384): for each d-chunk kd,
      gate[t, a-slice] += xT(kd) .T-matmul w1_tile(kd)   (PSUM, N=512)
      up[t, a-slice]   += xT(kd) .T-matmul w3_tile(kd)
    (x is the stationary operand -- loaded once per kd, amortized over
    both matmuls; the moving operand is the fp8 weight tile at N=512,
    so the PE streams near peak instead of reloading weights per MM).
    Then silu(scale=1/64)*up -> h[t, a-slice] (bf16), and PE-transpose
    128-col chunks into hT[a, t] for pass 2.
  pass 2: down[t, d-quarter] += hT(ka).T-matmul w2_tile(ka, q),
    accumulated over all 23 ka in 2 PSUM banks per 1024-wide quarter,
    then copied to SBUF and DMA'd out as bf16 partials.

DMA rings: w13 + hT transposes + out on SP (sync), w2 + x on ACT
(scalar) so w2 tiles prefetch during pass 1 without head-of-line
blocking behind w13.
"""

import json
import os

import ml_dtypes
import numpy as np

T = 128          # tokens
D = 4096         # hidden dim
ACTIVE = 11468   # sparsity-selected neurons per expert
NCORES = 8
ASH = ACTIVE // 4        # 2867 active rows per core
NKA = 23                 # 128-row a-chunks per core
APAD = NKA * 128         # 2944
NKD = D // 128           # 32 d-chunks
JW_LAST = ASH - (NKA - 1) * 128  # 51 useful rows in the last a-chunk
NSL = 6                  # a-slices in pass 1
SLW = 512                # slice width (last slice: 384)
SLW_LAST = APAD - 5 * SLW  # 384
NQ = 4                   # pass-2 d-quarters
QW = D // NQ             # 1024

WSCALE = 64.0            # fp8 weight scale for w1/w3

BF16 = ml_dtypes.bfloat16
E3M4 = ml_dtypes.float8_e3m4

_EVENTSEM_CAP = 2


def _split_multi_waits(bir_json: bytes) -> bytes:
    """Hoist excess per-instruction sync waits into standalone waits.

    The axon-path walrus build accepts at most 1 sync-wait command per
    instruction (2 for EventSemaphore); Tile's wait assigner can emit
    more. Extra waits become wait-only EventSemaphore instructions
    inserted just before the offender on the same engine stream, which
    preserves semantics (the engine would have blocked there anyway).
    """
    d = json.loads(bir_json)
    for func in d.get("functions", []):
        for blk in func.get("blocks", []):
            out = []
            for inst in blk.get("instructions", []):
                sync = inst.get("sync_info")
                waits = (sync or {}).get("on_wait") or []
                cap = _EVENTSEM_CAP if inst.get("opcode") == "EventSemaphore" else 1
                if len(waits) > cap:
                    extra, keep = waits[:-cap], waits[-cap:]
                    for j in range(0, len(extra), _EVENTSEM_CAP):
                        w_inst = {
                            "engine": inst["engine"],
                            "ins": [],
                            "name": f"{inst['name']}-hw{j}",
                            "opcode": "EventSemaphore",
                            "outs": [],
                            "sync_info": {
                                "on_update": [],
                                "on_wait": extra[j : j + _EVENTSEM_CAP],
                            },
                        }
                        if "debug" in inst:
                            w_inst["debug"] = inst["debug"]
                        out.append(w_inst)
                    sync["on_wait"] = keep
                out.append(inst)
            blk["instructions"] = out
    return json.dumps(d).encode()


def _hoist_head_dmas(bir_json: bytes, max_hoist: int = 3) -> bytes:
    """Move the leading wait-free SP DMACopies into the preamble block.

    Tile's prologue (engine reg-init + const memsets + all-engine
    barrier) takes ~7us before the first dma_start issues, leaving the
    HBM wire idle. The first input DMAs have no waits and their
    destination tiles are disjoint from everything the preamble writes
    (asserted below), so issuing them before the barrier is safe: HWDGE
    keeps per-engine FIFO order and their sem increments are only ever
    waited on with >= thresholds.
    """
    d = json.loads(bir_json)
    for func in d.get("functions", []):
        blocks = func.get("blocks", [])
        if len(blocks) < 2:
            continue
        main, tile_blk = blocks[0], blocks[1]
        if main.get("name") != "main" or not tile_blk.get("name", "").startswith(
            "tile_context"
        ):
            continue
        # preamble must write only const tiles, else hoisting is unsafe
        pre_outs = {
            o.get("memref")
            for inst in main["instructions"]
            for o in inst.get("outs", [])
            if isinstance(o, dict)
        }
        if any(m and not m.startswith("const-") for m in pre_outs):
            continue
        hoisted = []
        remaining = []
        for inst in tile_blk["instructions"]:
            if (
                len(hoisted) < max_hoist
                and inst.get("engine") == "SP"
                and inst.get("opcode") == "DMACopy"
                and not ((inst.get("sync_info") or {}).get("on_wait"))
            ):
                hoisted.append(inst)
            else:
                remaining.append(inst)
        if not hoisted:
            continue
        ins_at = next(
            (
                i
                for i, inst in enumerate(main["instructions"])
                if inst.get("engine") == "SP" and inst.get("opcode") == "Drain"
            ),
            None,
        )
        if ins_at is None:
            continue
        main["instructions"][ins_at:ins_at] = hoisted
        tile_blk["instructions"] = remaining
    return json.dumps(d).encode()


def _install_wait_split():
    import concourse.bass2jax as b2j
    import concourse.bass_utils as bu

    if getattr(bu.compile_bir_kernel, "_wait_split", False):
        return
    orig = bu.compile_bir_kernel

    def compile_with_split(bir_json, tmpdir, neff_name="file.neff"):
        return orig(_split_multi_waits(_hoist_head_dmas(bir_json)), tmpdir, neff_name)

    compile_with_split._wait_split = True
    bu.compile_bir_kernel = compile_with_split
    if getattr(b2j, "compile_bir_kernel", None) is orig:
        b2j.compile_bir_kernel = compile_with_split


_program = None


def _build_program():
    """Build the single-core Bass/Tile program (same program on all 8 cores)."""
    import concourse.bass as bass
    import concourse.mybir as mybir
    from concourse.tile import TileContext

    f32 = mybir.dt.float32
    bf16 = mybir.dt.bfloat16
    fp8e3 = mybir.dt.float8e3
    Silu = mybir.ActivationFunctionType.Silu

    nc = bass.Bass()
    xb = nc.declare_dram_parameter("xb", [128, D], bf16, isOutput=False)
    w13a = nc.declare_dram_parameter(
        "w13a", [NSL - 1, NKD // 4, 128, 8 * SLW], fp8e3, isOutput=False
    )
    w13b = nc.declare_dram_parameter(
        "w13b", [NKD // 4, 128, 8 * SLW_LAST], fp8e3, isOutput=False
    )
    w2q = nc.declare_dram_parameter("w2q", [NQ, NKA, 128, QW], bf16, isOutput=False)
    out = nc.declare_dram_parameter("out", [T, D], bf16, isOutput=True)

    def slw_of(s):
        return SLW_LAST if s == NSL - 1 else SLW

    def jw_of(ka):
        return JW_LAST if ka == NKA - 1 else 128

    with TileContext(nc) as tc:
        with (
            tc.tile_pool(name="singles", bufs=1) as singles,
            tc.tile_pool(name="w13p", bufs=5) as w13p,
            tc.tile_pool(name="w2p", bufs=16) as w2p,
            tc.tile_pool(name="gap", bufs=2) as gap,
            tc.tile_pool(name="hp", bufs=2) as hp,
            tc.tile_pool(name="outp", bufs=2) as outp,
            tc.tile_pool(name="psum_ug", bufs=2, space="PSUM") as psum_ug,
            tc.tile_pool(name="psum_o", bufs=2, space="PSUM") as psum_o,
        ):
            # x on the ACT HWDGE ring so it overlaps w13[0] on SP's
            xb_s = singles.tile([128, D], bf16)
            nc.scalar.dma_start(out=xb_s, in_=xb[:, :])
            hT = singles.tile([128, APAD], bf16)

            # pass 1: gate/up in [t, a] layout, x stationary, fp8 w13 moving.
            # w13 arrives in 4-kd batches (512 KB, 4 KB partition lines).
            for s in range(NSL):
                sw = slw_of(s)
                gate_ps = psum_ug.tile([128, sw], f32, name=f"g{s}", tag="gate")
                up_ps = psum_ug.tile([128, sw], f32, name=f"u{s}", tag="up")
                for g in range(NKD // 4):
                    w13t = w13p.tile([128, 8 * sw], fp8e3)
                    if s < NSL - 1:
                        nc.sync.dma_start(out=w13t, in_=w13a[s, g, :, :])
                    else:
                        nc.sync.dma_start(out=w13t, in_=w13b[g, :, :])
                    for m in range(4):
                        kd = 4 * g + m
                        lhsT = xb_s[:, kd * 128 : (kd + 1) * 128]
                        nc.tensor.matmul(
                            gate_ps,
                            lhsT,
                            w13t[:, m * 2 * sw : m * 2 * sw + sw],
                            start=(kd == 0),
                            stop=(kd == NKD - 1),
                        )
                        nc.tensor.matmul(
                            up_ps,
                            lhsT,
                            w13t[:, m * 2 * sw + sw : (m + 1) * 2 * sw],
                            start=(kd == 0),
                            stop=(kd == NKD - 1),
                        )
                ga = gap.tile([128, sw], f32)
                nc.scalar.activation(
                    out=ga, in_=gate_ps, func=Silu, scale=1.0 / WSCALE
                )
                hsl = hp.tile([128, sw], bf16)
                nc.vector.tensor_mul(out=hsl, in0=ga, in1=up_ps)
                # hT chunks via the DMA XBAR transpose (off the PE and off
                # the HBM wire; SBUF->SBUF on the SP ring)
                for c in range(sw // 128):
                    ka = s * 4 + c
                    nc.sync.dma_start(
                        out=hT[:, ka * 128 : (ka + 1) * 128],
                        in_=hsl[:, c * 128 : (c + 1) * 128],
                        transpose=True,
                    )

            # pass 2: down projection in 4 d-quarters; each quarter
            # accumulates across all ka in 2 PSUM banks, then its output
            # copy + DMA overlaps the next quarter's compute.
            for q in range(NQ):
                ops = psum_o.tile([128, QW], f32, name=f"o{q}", tag="od")
                for ka in range(NKA):
                    jw = jw_of(ka)
                    w2t = w2p.tile([128, QW], bf16)
                    nc.scalar.dma_start(out=w2t[:jw], in_=w2q[q, ka, :jw, :])
                    lhsT = hT[:jw, ka * 128 : (ka + 1) * 128]
                    for b in range(2):
                        nc.tensor.matmul(
                            ops[:, b * 512 : (b + 1) * 512],
                            lhsT,
                            w2t[:jw, b * 512 : (b + 1) * 512],
                            start=(ka == 0),
                            stop=(ka == NKA - 1),
                        )
                oth = outp.tile([T, QW], bf16, name=f"oth{q}", tag="oth")
                nc.vector.tensor_copy(out=oth, in_=ops)
                # output store on the SP ring (idle once w13 is done):
                # doesn't queue behind the remaining w2 transfers on ACT
                nc.sync.dma_start(out=out[:, q * QW : (q + 1) * QW], in_=oth)

    return nc


def _pack_w13(w1s: np.ndarray, w3s: np.ndarray):
    """[ASH, D] f32 pair -> ([NSL-1, 8, 128, 8*SLW], [8, 128, 8*SLW_LAST])
    e3m4 blobs, scaled by WSCALE, batched 4 d-chunks per transfer.

    blob[s, g, p, m*2*sw + w*sw + j] = w{1,3}s[s*SLW + j, (4g+m)*128 + p].
    """
    padded = np.zeros((2, NSL * SLW, D), dtype=E3M4)
    padded[0, :ASH] = (w1s * np.float32(WSCALE)).astype(E3M4)
    padded[1, :ASH] = (w3s * np.float32(WSCALE)).astype(E3M4)
    # [2, NSL, SLW, NKD, 128] -> [s, kd, p, w, j]
    r = padded.reshape(2, NSL, SLW, NKD, 128).transpose(1, 3, 4, 0, 2)
    # [s, g, m, p, w, j] -> [s, g, p, m, w, j]
    r = r.reshape(NSL, NKD // 4, 4, 128, 2, SLW).transpose(0, 1, 3, 2, 4, 5)
    a = np.ascontiguousarray(r[: NSL - 1]).reshape(NSL - 1, NKD // 4, 128, 8 * SLW)
    b = np.ascontiguousarray(r[NSL - 1, :, :, :, :, :SLW_LAST]).reshape(
        NKD // 4, 128, 8 * SLW_LAST
    )
    return a, b


def _pack_w2(w2s: np.ndarray, scale: float) -> np.ndarray:
    """[ASH, D] f32 -> [NQ, NKA, 128, QW] bf16 blob, scale folded in."""
    p = np.zeros((APAD, D), dtype=BF16)
    p[:ASH] = (w2s * np.float32(scale)).astype(BF16)
    return np.ascontiguousarray(p.reshape(NKA, 128, NQ, QW).transpose(2, 0, 1, 3))


def _pack_x(x: np.ndarray) -> np.ndarray:
    """[T, D] f32 -> [128, D] bf16: xb[p, kd*128 + t] = x[t, kd*128 + p]."""
    return (
        x.astype(BF16).reshape(T, NKD, 128).transpose(2, 1, 0).reshape(128, NKD * T)
    )


def make_in_maps(
    hidden_states,
    expert_weights,
    expert_ids,
    w1_e0,
    w3_e0,
    w2_e0,
    w1_e1,
    w3_e1,
    w2_e1,
):
    ids = np.asarray(expert_ids).reshape(-1)
    ew = np.asarray(expert_weights, dtype=np.float32).reshape(-1)
    if int(ids[0]) != 0:
        ew = ew[::-1]

    xb = _pack_x(np.asarray(hidden_states, dtype=np.float32))
    w1 = (np.asarray(w1_e0, np.float32), np.asarray(w1_e1, np.float32))
    w3 = (np.asarray(w3_e0, np.float32), np.asarray(w3_e1, np.float32))
    w2 = (np.asarray(w2_e0, np.float32), np.asarray(w2_e1, np.float32))

    in_maps = []
    for core in range(NCORES):
        e, r = divmod(core, 4)
        rows = slice(r * ASH, (r + 1) * ASH)
        w13a, w13b = _pack_w13(w1[e][rows], w3[e][rows])
        # h is scaled by WSCALE (up de-scale folded here), so w2 gets
        # ew / WSCALE.
        w2blob = _pack_w2(w2[e][rows], float(ew[e]) / WSCALE)
        in_maps.append(
            {
                "xb": xb,
                "w13a": w13a,
                "w13b": w13b,
                "w2q": w2blob,
            }
        )
    return in_maps


LAST_RESULT = None


def kernel(**inputs) -> np.ndarray:
    global _program, LAST_RESULT
    _install_wait_split()
    from concourse.bass_utils import run_bass_kernel_spmd

    if _program is None:
        _program = _build_program()
        # Apply the BIR transforms at serialization time so the embedded
        # ant_bir payload (the compile-cache key) reflects them. Both
        # transforms are idempotent, so compile_bir_kernel re-applying
        # them is harmless.
        orig_tjb = _program.to_json_bytes

        def _tjb():
            return _split_multi_waits(_hoist_head_dmas(orig_tjb()))

        _program.to_json_bytes = _tjb

    in_maps = make_in_maps(**inputs)
    res = run_bass_kernel_spmd(
        _program,
        in_maps,
        core_ids=list(range(NCORES)),
        trace=bool(int(os.environ.get("KERNEL_TRACE", "0"))),
    )
    LAST_RESULT = res
    out = np.zeros((T, D), dtype=np.float32)
    for r in res.results:
        out += np.asarray(r["out"]).astype(np.float32)
    return out


# revision 32
# speedup vs baseline: 1.3462x; 1.3367x over previous
"""Trainium2 Bass kernel for nn_CachedMLP (2-expert dense MoE MLP).

Computation (reference):
    ew = expert_weights, swapped if expert_ids[0] != 0
    for e in {0,1}:  down_e = (silu(x @ w1_e.T) * (x @ w3_e.T)) @ w2_e
    out = down_0 * ew[0] + down_1 * ew[1]

Sharding: expert-parallel x tensor-parallel. Core c handles expert c//4
and rows [r*2867, (r+1)*2867) of that expert's w1/w3/w2 (r = c%4),
zero-padded to 2944 = 23*128. ew[e] (and the fp8 weight scale) is
folded into w2 on the host, so the sum of the 8 per-core partial
outputs is the final result.

Precision: w1/w3 are stored as fp8-E3M4 (scaled x64 on host; the PE
upconverts exactly, and silu de-scales via activation's scale=1/64);
w2, x, h stay bf16. Measured end-to-end rel err ~1.6% (tolerance 2e-2).
This halves the w13 HBM traffic: per-core reads drop 71.5 -> ~48 MB,
and the kernel is HBM-wire bound.

Device kernel per core:
  pass 1 over 6 a-slices (512 wide, last 384): for each d-chunk kd,
      gate[t, a-slice] += xT(kd) .T-matmul w1_tile(kd)   (PSUM, N=512)
      up[t, a-slice]   += xT(kd) .T-matmul w3_tile(kd)
    (x is the stationary operand -- loaded once per kd, amortized over
    both matmuls; the moving operand is the fp8 weight tile at N=512,
    so the PE streams near peak instead of reloading weights per MM).
    Then silu(scale=1/64)*up -> h[t, a-slice] (bf16), and PE-transpose
    128-col chunks into hT[a, t] for pass 2.
  pass 2: down[t, d-quarter] += hT(ka).T-matmul w2_tile(ka, q),
    accumulated over all 23 ka in 2 PSUM banks per 1024-wide quarter,
    then copied to SBUF and DMA'd out as bf16 partials.

DMA rings: w13 + hT transposes + out on SP (sync), w2 + x on ACT
(scalar) so w2 tiles prefetch during pass 1 without head-of-line
blocking behind w13.
"""

import json
import os

import ml_dtypes
import numpy as np

T = 128          # tokens
D = 4096         # hidden dim
ACTIVE = 11468   # sparsity-selected neurons per expert
NCORES = 8
ASH = ACTIVE // 4        # 2867 active rows per core
NKA = 23                 # 128-row a-chunks per core
APAD = NKA * 128         # 2944
NKD = D // 128           # 32 d-chunks
JW_LAST = ASH - (NKA - 1) * 128  # 51 useful rows in the last a-chunk
NSL = 6                  # a-slices in pass 1
SLW = 512                # slice width (last slice: 384)
SLW_LAST = APAD - 5 * SLW  # 384
NQ = 4                   # pass-2 d-quarters
QW = D // NQ             # 1024

WSCALE = 64.0            # fp8 weight scale for w1/w3
K8 = 12                  # leading 128-row a-chunks of w2 stored e3m4
KB16 = NKA - K8          # trailing chunks kept bf16

BF16 = ml_dtypes.bfloat16
E3M4 = ml_dtypes.float8_e3m4

_EVENTSEM_CAP = 2


def _split_multi_waits(bir_json: bytes) -> bytes:
    """Hoist excess per-instruction sync waits into standalone waits.

    The axon-path walrus build accepts at most 1 sync-wait command per
    instruction (2 for EventSemaphore); Tile's wait assigner can emit
    more. Extra waits become wait-only EventSemaphore instructions
    inserted just before the offender on the same engine stream, which
    preserves semantics (the engine would have blocked there anyway).
    """
    d = json.loads(bir_json)
    for func in d.get("functions", []):
        for blk in func.get("blocks", []):
            out = []
            for inst in blk.get("instructions", []):
                sync = inst.get("sync_info")
                waits = (sync or {}).get("on_wait") or []
                cap = _EVENTSEM_CAP if inst.get("opcode") == "EventSemaphore" else 1
                if len(waits) > cap:
                    extra, keep = waits[:-cap], waits[-cap:]
                    for j in range(0, len(extra), _EVENTSEM_CAP):
                        w_inst = {
                            "engine": inst["engine"],
                            "ins": [],
                            "name": f"{inst['name']}-hw{j}",
                            "opcode": "EventSemaphore",
                            "outs": [],
                            "sync_info": {
                                "on_update": [],
                                "on_wait": extra[j : j + _EVENTSEM_CAP],
                            },
                        }
                        if "debug" in inst:
                            w_inst["debug"] = inst["debug"]
                        out.append(w_inst)
                    sync["on_wait"] = keep
                out.append(inst)
            blk["instructions"] = out
    return json.dumps(d).encode()


def _hoist_head_dmas(bir_json: bytes, max_hoist: int = 3) -> bytes:
    """Move the leading wait-free SP DMACopies into the preamble block.

    Tile's prologue (engine reg-init + const memsets + all-engine
    barrier) takes ~7us before the first dma_start issues, leaving the
    HBM wire idle. The first input DMAs have no waits and their
    destination tiles are disjoint from everything the preamble writes
    (asserted below), so issuing them before the barrier is safe: HWDGE
    keeps per-engine FIFO order and their sem increments are only ever
    waited on with >= thresholds.
    """
    d = json.loads(bir_json)
    for func in d.get("functions", []):
        blocks = func.get("blocks", [])
        if len(blocks) < 2:
            continue
        main, tile_blk = blocks[0], blocks[1]
        if main.get("name") != "main" or not tile_blk.get("name", "").startswith(
            "tile_context"
        ):
            continue
        # preamble must write only const tiles, else hoisting is unsafe
        pre_outs = {
            o.get("memref")
            for inst in main["instructions"]
            for o in inst.get("outs", [])
            if isinstance(o, dict)
        }
        if any(m and not m.startswith("const-") for m in pre_outs):
            continue
        hoisted = []
        remaining = []
        for inst in tile_blk["instructions"]:
            if (
                len(hoisted) < max_hoist
                and inst.get("engine") == "SP"
                and inst.get("opcode") == "DMACopy"
                and not ((inst.get("sync_info") or {}).get("on_wait"))
            ):
                hoisted.append(inst)
            else:
                remaining.append(inst)
        if not hoisted:
            continue
        ins_at = next(
            (
                i
                for i, inst in enumerate(main["instructions"])
                if inst.get("engine") == "SP" and inst.get("opcode") == "Drain"
            ),
            None,
        )
        if ins_at is None:
            continue
        main["instructions"][ins_at:ins_at] = hoisted
        tile_blk["instructions"] = remaining
    return json.dumps(d).encode()


def _install_wait_split():
    import concourse.bass2jax as b2j
    import concourse.bass_utils as bu

    if getattr(bu.compile_bir_kernel, "_wait_split", False):
        return
    orig = bu.compile_bir_kernel

    def compile_with_split(bir_json, tmpdir, neff_name="file.neff"):
        return orig(_split_multi_waits(_hoist_head_dmas(bir_json)), tmpdir, neff_name)

    compile_with_split._wait_split = True
    bu.compile_bir_kernel = compile_with_split
    if getattr(b2j, "compile_bir_kernel", None) is orig:
        b2j.compile_bir_kernel = compile_with_split


_program = None


def _build_program():
    """Build the single-core Bass/Tile program (same program on all 8 cores)."""
    import concourse.bass as bass
    import concourse.mybir as mybir
    from concourse.tile import TileContext

    f32 = mybir.dt.float32
    bf16 = mybir.dt.bfloat16
    fp8e3 = mybir.dt.float8e3
    Silu = mybir.ActivationFunctionType.Silu

    nc = bass.Bass()
    xb = nc.declare_dram_parameter("xb", [128, D], bf16, isOutput=False)
    ident = nc.declare_dram_parameter("ident", [128, 128], bf16, isOutput=False)
    w13a = nc.declare_dram_parameter(
        "w13a", [NSL - 1, NKD // 8, 128, 16 * SLW], fp8e3, isOutput=False
    )
    w13b = nc.declare_dram_parameter(
        "w13b", [NKD // 8, 128, 16 * SLW_LAST], fp8e3, isOutput=False
    )
    w2q8 = nc.declare_dram_parameter(
        "w2q8", [NQ, K8 // 2, 128, 2 * QW], fp8e3, isOutput=False
    )
    w2qb = nc.declare_dram_parameter(
        "w2qb", [NQ, KB16, 128, QW], bf16, isOutput=False
    )
    oscale = nc.declare_dram_parameter("oscale", [128, 1], f32, isOutput=False)
    out = nc.declare_dram_parameter("out", [T, D], bf16, isOutput=True)

    def slw_of(s):
        return SLW_LAST if s == NSL - 1 else SLW

    def jw_of(ka):
        return JW_LAST if ka == NKA - 1 else 128

    with TileContext(nc) as tc:
        with (
            tc.tile_pool(name="singles", bufs=1) as singles,
            tc.tile_pool(name="w13p", bufs=4) as w13p,
            tc.tile_pool(name="w2p", bufs=40) as w2p,
            tc.tile_pool(name="gap", bufs=2) as gap,
            tc.tile_pool(name="hp", bufs=2) as hp,
            tc.tile_pool(name="outp", bufs=2) as outp,
            tc.tile_pool(name="psum_ug", bufs=2, space="PSUM") as psum_ug,
            tc.tile_pool(name="psum_t", bufs=2, space="PSUM") as psum_t,
            tc.tile_pool(name="psum_o", bufs=1, space="PSUM") as psum_o,
        ):
            # x on the ACT HWDGE ring so it overlaps w13[0] on SP's
            xb_s = singles.tile([128, D], bf16)
            nc.scalar.dma_start(out=xb_s, in_=xb[:, :])
            osc_s = singles.tile([128, 1], f32)
            nc.scalar.dma_start(out=osc_s, in_=oscale[:, :])
            id_s = singles.tile([128, 128], bf16)
            nc.scalar.dma_start(out=id_s, in_=ident[:, :])
            hT = singles.tile([128, APAD], bf16)

            # pass 1: gate/up in [t, a] layout, x stationary, fp8 w13 moving.
            # w13 arrives in 4-kd batches (512 KB, 4 KB partition lines).
            for s in range(NSL):
                sw = slw_of(s)
                gate_ps = psum_ug.tile([128, sw], f32, name=f"g{s}", tag="gate")
                up_ps = psum_ug.tile([128, sw], f32, name=f"u{s}", tag="up")
                for g in range(NKD // 8):
                    w13t = w13p.tile([128, 16 * sw], fp8e3)
                    if s < NSL - 1:
                        nc.sync.dma_start(out=w13t, in_=w13a[s, g, :, :])
                    else:
                        nc.sync.dma_start(out=w13t, in_=w13b[g, :, :])
                    for m in range(8):
                        kd = 8 * g + m
                        lhsT = xb_s[:, kd * 128 : (kd + 1) * 128]
                        nc.tensor.matmul(
                            gate_ps,
                            lhsT,
                            w13t[:, m * 2 * sw : m * 2 * sw + sw],
                            start=(kd == 0),
                            stop=(kd == NKD - 1),
                        )
                        nc.tensor.matmul(
                            up_ps,
                            lhsT,
                            w13t[:, m * 2 * sw + sw : (m + 1) * 2 * sw],
                            start=(kd == 0),
                            stop=(kd == NKD - 1),
                        )
                ga = gap.tile([128, sw], f32)
                nc.scalar.activation(
                    out=ga, in_=gate_ps, func=Silu, scale=1.0 / WSCALE
                )
                hsl = hp.tile([128, sw], bf16)
                nc.vector.tensor_mul(out=hsl, in0=ga, in1=up_ps)
                # hT chunks via PE transpose + DVE copy. NOT the DMA XBAR:
                # a 128x128 XBAR transpose costs ~1.25us of DMA-engine
                # time (26 GB/s), and the single shared DMA engine is the
                # kernel's scarcest resource; 375ns on the PE is cheaper.
                for c in range(sw // 128):
                    ka = s * 4 + c
                    tp = psum_t.tile([128, 128], bf16, name=f"t{ka}", tag="tp")
                    nc.tensor.transpose(tp, hsl[:, c * 128 : (c + 1) * 128], id_s)
                    nc.vector.tensor_copy(
                        out=hT[:, ka * 128 : (ka + 1) * 128], in_=tp
                    )

            # pass 2: down projection in 4 d-quarters; each quarter
            # accumulates across all ka in 2 PSUM banks, then its output
            # de-scale copy + DMA overlaps the next quarter's compute.
            # ka < K8 chunks are e3m4 (2-ka pairs per 256 KB transfer),
            # the rest bf16; all pre-scaled by S2, undone by oscale.
            for q in range(NQ):
                ops = psum_o.tile([128, QW], f32, name=f"o{q}", tag="od")
                for kp in range(K8 // 2):
                    w2t = w2p.tile([128, 2 * QW], fp8e3)
                    nc.scalar.dma_start(out=w2t, in_=w2q8[q, kp, :, :])
                    for hf in range(2):
                        ka = 2 * kp + hf
                        lhsT = hT[:, ka * 128 : (ka + 1) * 128]
                        for b in range(2):
                            nc.tensor.matmul(
                                ops[:, b * 512 : (b + 1) * 512],
                                lhsT,
                                w2t[:, hf * QW + b * 512 : hf * QW + (b + 1) * 512],
                                start=(ka == 0),
                                stop=False,
                            )
                for kb in range(KB16):
                    ka = K8 + kb
                    jw = jw_of(ka)
                    w2t = w2p.tile([128, QW], bf16)
                    nc.scalar.dma_start(out=w2t[:jw], in_=w2qb[q, kb, :jw, :])
                    lhsT = hT[:jw, ka * 128 : (ka + 1) * 128]
                    for b in range(2):
                        nc.tensor.matmul(
                            ops[:, b * 512 : (b + 1) * 512],
                            lhsT,
                            w2t[:jw, b * 512 : (b + 1) * 512],
                            start=False,
                            stop=(ka == NKA - 1),
                        )
                oth = outp.tile([T, QW], bf16, name=f"oth{q}", tag="oth")
                nc.vector.tensor_scalar_mul(out=oth, in0=ops, scalar1=osc_s)
                # output store on the SP ring (idle once w13 is done):
                # doesn't queue behind the remaining w2 transfers on ACT
                nc.sync.dma_start(out=out[:, q * QW : (q + 1) * QW], in_=oth)

    return nc


def _pack_w13(w1s: np.ndarray, w3s: np.ndarray):
    """[ASH, D] f32 pair -> ([NSL-1, 4, 128, 16*SLW], [4, 128, 16*SLW_LAST])
    e3m4 blobs, scaled by WSCALE, batched 8 d-chunks per 1 MiB transfer
    (8 KB partition lines, the shape that sustains full DMA throughput).

    blob[s, g, p, m*2*sw + w*sw + j] = w{1,3}s[s*SLW + j, (8g+m)*128 + p].
    """
    padded = np.zeros((2, NSL * SLW, D), dtype=E3M4)
    padded[0, :ASH] = (w1s * np.float32(WSCALE)).astype(E3M4)
    padded[1, :ASH] = (w3s * np.float32(WSCALE)).astype(E3M4)
    # [2, NSL, SLW, NKD, 128] -> [s, kd, p, w, j]
    r = padded.reshape(2, NSL, SLW, NKD, 128).transpose(1, 3, 4, 0, 2)
    # [s, g, m, p, w, j] -> [s, g, p, m, w, j]
    r = r.reshape(NSL, NKD // 8, 8, 128, 2, SLW).transpose(0, 1, 3, 2, 4, 5)
    a = np.ascontiguousarray(r[: NSL - 1]).reshape(NSL - 1, NKD // 8, 128, 16 * SLW)
    b = np.ascontiguousarray(r[NSL - 1, :, :, :, :, :SLW_LAST]).reshape(
        NKD // 8, 128, 16 * SLW_LAST
    )
    return a, b


def _pack_w2(w2s: np.ndarray, scale: float):
    """[ASH, D] f32 -> ([NQ, K8/2, 128, 2*QW] e3m4, [NQ, KB16, 128, QW] bf16).

    scale (= ew * S2 / WSCALE) is folded in; rows < K8*128 go e3m4 in
    2-ka pairs, the rest bf16.
    """
    v = w2s * np.float32(scale)
    n8 = K8 * 128
    e = v[:n8].astype(E3M4)
    a = np.ascontiguousarray(
        e.reshape(K8 // 2, 2, 128, NQ, QW).transpose(3, 0, 2, 1, 4)
    ).reshape(NQ, K8 // 2, 128, 2 * QW)
    p = np.zeros((KB16 * 128, D), dtype=BF16)
    p[: ASH - n8] = v[n8:].astype(BF16)
    b = np.ascontiguousarray(p.reshape(KB16, 128, NQ, QW).transpose(2, 0, 1, 3))
    return a, b


def _pack_x(x: np.ndarray) -> np.ndarray:
    """[T, D] f32 -> [128, D] bf16: xb[p, kd*128 + t] = x[t, kd*128 + p]."""
    return (
        x.astype(BF16).reshape(T, NKD, 128).transpose(2, 1, 0).reshape(128, NKD * T)
    )


def make_in_maps(
    hidden_states,
    expert_weights,
    expert_ids,
    w1_e0,
    w3_e0,
    w2_e0,
    w1_e1,
    w3_e1,
    w2_e1,
):
    ids = np.asarray(expert_ids).reshape(-1)
    ew = np.asarray(expert_weights, dtype=np.float32).reshape(-1)
    if int(ids[0]) != 0:
        ew = ew[::-1]

    xb = _pack_x(np.asarray(hidden_states, dtype=np.float32))
    ident = np.eye(128, dtype=BF16)
    w1 = (np.asarray(w1_e0, np.float32), np.asarray(w1_e1, np.float32))
    w3 = (np.asarray(w3_e0, np.float32), np.asarray(w3_e1, np.float32))
    w2 = (np.asarray(w2_e0, np.float32), np.asarray(w2_e1, np.float32))

    in_maps = []
    for core in range(NCORES):
        e, r = divmod(core, 4)
        rows = slice(r * ASH, (r + 1) * ASH)
        w13a, w13b = _pack_w13(w1[e][rows], w3[e][rows])
        # h is scaled by WSCALE (up de-scale folded here), so w2 gets
        # ew / WSCALE, plus S2 to center the e3m4 chunks in range
        # (sigma_eff ~ 2); S2 is undone by oscale on the output copy.
        ewe = float(ew[e])
        s2 = 2.0 ** np.round(np.log2(6400.0 / max(abs(ewe), 1e-6)))
        w2q8, w2qb = _pack_w2(w2[e][rows], ewe * s2 / WSCALE)
        in_maps.append(
            {
                "xb": xb,
                "ident": ident,
                "w13a": w13a,
                "w13b": w13b,
                "w2q8": w2q8,
                "w2qb": w2qb,
                "oscale": np.full((128, 1), 1.0 / s2, dtype=np.float32),
            }
        )
    return in_maps


LAST_RESULT = None


def kernel(**inputs) -> np.ndarray:
    global _program, LAST_RESULT
    _install_wait_split()
    from concourse.bass_utils import run_bass_kernel_spmd

    if _program is None:
        _program = _build_program()
        # Apply the BIR transforms at serialization time so the embedded
        # ant_bir payload (the compile-cache key) reflects them. Both
        # transforms are idempotent, so compile_bir_kernel re-applying
        # them is harmless.
        orig_tjb = _program.to_json_bytes

        def _tjb():
            return _split_multi_waits(_hoist_head_dmas(orig_tjb()))

        _program.to_json_bytes = _tjb

    in_maps = make_in_maps(**inputs)
    res = run_bass_kernel_spmd(
        _program,
        in_maps,
        core_ids=list(range(NCORES)),
        trace=bool(int(os.environ.get("KERNEL_TRACE", "0"))),
    )
    LAST_RESULT = res
    out = np.zeros((T, D), dtype=np.float32)
    for r in res.results:
        out += np.asarray(r["out"]).astype(np.float32)
    return out


# revision 41
# speedup vs baseline: 1.3832x; 1.0275x over previous
"""Trainium2 Bass kernel for nn_CachedMLP (2-expert dense MoE MLP).

Computation (reference):
    ew = expert_weights, swapped if expert_ids[0] != 0
    for e in {0,1}:  down_e = (silu(x @ w1_e.T) * (x @ w3_e.T)) @ w2_e
    out = down_0 * ew[0] + down_1 * ew[1]

Sharding: expert-parallel x tensor-parallel. Core c handles expert c//4
and rows [r*2867, (r+1)*2867) of that expert's w1/w3/w2 (r = c%4),
zero-padded to 2944 = 23*128. ew[e] (and the fp8 weight scale) is
folded into w2 on the host, so the sum of the 8 per-core partial
outputs is the final result.

Precision: w1/w3 are stored as fp8-E3M4 (scaled x64 on host; the PE
upconverts exactly, and silu de-scales via activation's scale=1/64);
w2, x, h stay bf16. Measured end-to-end rel err ~1.6% (tolerance 2e-2).
This halves the w13 HBM traffic: per-core reads drop 71.5 -> ~48 MB,
and the kernel is HBM-wire bound.

Device kernel per core:
  pass 1 over 6 a-slices (512 wide, last 384): for each d-chunk kd,
      gate[t, a-slice] += xT(kd) .T-matmul w1_tile(kd)   (PSUM, N=512)
      up[t, a-slice]   += xT(kd) .T-matmul w3_tile(kd)
    (x is the stationary operand -- loaded once per kd, amortized over
    both matmuls; the moving operand is the fp8 weight tile at N=512,
    so the PE streams near peak instead of reloading weights per MM).
    Then silu(scale=1/64)*up -> h[t, a-slice] (bf16), and PE-transpose
    128-col chunks into hT[a, t] for pass 2.
  pass 2: down[t, d-quarter] += hT(ka).T-matmul w2_tile(ka, q),
    accumulated over all 23 ka in 2 PSUM banks per 1024-wide quarter,
    then copied to SBUF and DMA'd out as bf16 partials.

DMA rings: w13 + hT transposes + out on SP (sync), w2 + x on ACT
(scalar) so w2 tiles prefetch during pass 1 without head-of-line
blocking behind w13.
"""

import json
import os

import ml_dtypes
import numpy as np

T = 128          # tokens
D = 4096         # hidden dim
ACTIVE = 11468   # sparsity-selected neurons per expert
NCORES = 8
ASH = ACTIVE // 4        # 2867 active rows per core
NKA = 23                 # 128-row a-chunks per core
APAD = NKA * 128         # 2944
NKD = D // 128           # 32 d-chunks
JW_LAST = ASH - (NKA - 1) * 128  # 51 useful rows in the last a-chunk
NSL = 6                  # a-slices in pass 1
SLW = 512                # slice width (last slice: 384)
SLW_LAST = APAD - 5 * SLW  # 384
NQ = 4                   # pass-2 d-quarters
QW = D // NQ             # 1024

WSCALE = 64.0            # fp8 weight scale for w1/w3
K8 = 14                  # leading 128-row a-chunks of w2 stored e3m4
KB16 = NKA - K8          # trailing chunks kept bf16 (8 paired + last jw=51)

BF16 = ml_dtypes.bfloat16
E3M4 = ml_dtypes.float8_e3m4

_EVENTSEM_CAP = 2


def _split_multi_waits(bir_json: bytes) -> bytes:
    """Hoist excess per-instruction sync waits into standalone waits.

    The axon-path walrus build accepts at most 1 sync-wait command per
    instruction (2 for EventSemaphore); Tile's wait assigner can emit
    more. Extra waits become wait-only EventSemaphore instructions
    inserted just before the offender on the same engine stream, which
    preserves semantics (the engine would have blocked there anyway).
    """
    d = json.loads(bir_json)
    for func in d.get("functions", []):
        for blk in func.get("blocks", []):
            out = []
            for inst in blk.get("instructions", []):
                sync = inst.get("sync_info")
                waits = (sync or {}).get("on_wait") or []
                cap = _EVENTSEM_CAP if inst.get("opcode") == "EventSemaphore" else 1
                if len(waits) > cap:
                    extra, keep = waits[:-cap], waits[-cap:]
                    for j in range(0, len(extra), _EVENTSEM_CAP):
                        w_inst = {
                            "engine": inst["engine"],
                            "ins": [],
                            "name": f"{inst['name']}-hw{j}",
                            "opcode": "EventSemaphore",
                            "outs": [],
                            "sync_info": {
                                "on_update": [],
                                "on_wait": extra[j : j + _EVENTSEM_CAP],
                            },
                        }
                        if "debug" in inst:
                            w_inst["debug"] = inst["debug"]
                        out.append(w_inst)
                    sync["on_wait"] = keep
                out.append(inst)
            blk["instructions"] = out
    return json.dumps(d).encode()


def _hoist_head_dmas(bir_json: bytes, max_hoist: int = 4) -> bytes:
    """Move the leading wait-free SP DMACopies to the head of the preamble.

    Tile's prologue (engine reg-init + const memsets + all-engine
    barrier rounds) takes ~8.5us before the first dma_start issues,
    leaving the HBM wire idle. The first input DMAs have no waits and
    their destination tiles are disjoint from everything the preamble
    writes (asserted below), so issuing them at the very front of the
    SP stream is safe: DGE rings and DMA semaphores are initialized by
    the runtime at NEFF load (not by the preamble), HWDGE keeps
    per-engine FIFO order, and the sem increments are only ever waited
    on with >= thresholds.
    """
    d = json.loads(bir_json)
    for func in d.get("functions", []):
        blocks = func.get("blocks", [])
        if len(blocks) < 2:
            continue
        main, tile_blk = blocks[0], blocks[1]
        if main.get("name") != "main" or not tile_blk.get("name", "").startswith(
            "tile_context"
        ):
            continue
        # preamble must write only const tiles, else hoisting is unsafe
        pre_outs = {
            o.get("memref")
            for inst in main["instructions"]
            for o in inst.get("outs", [])
            if isinstance(o, dict)
        }
        if any(m and not m.startswith("const-") for m in pre_outs):
            continue
        hoisted = []
        remaining = []
        for inst in tile_blk["instructions"]:
            if (
                len(hoisted) < max_hoist
                and inst.get("engine") == "SP"
                and inst.get("opcode") == "DMACopy"
                and not ((inst.get("sync_info") or {}).get("on_wait"))
            ):
                hoisted.append(inst)
            else:
                remaining.append(inst)
        if not hoisted:
            continue
        main["instructions"][0:0] = hoisted
        tile_blk["instructions"] = remaining
    return json.dumps(d).encode()


def _install_wait_split():
    import concourse.bass2jax as b2j
    import concourse.bass_utils as bu

    if getattr(bu.compile_bir_kernel, "_wait_split", False):
        return
    orig = bu.compile_bir_kernel

    def compile_with_split(bir_json, tmpdir, neff_name="file.neff"):
        return orig(_split_multi_waits(_hoist_head_dmas(bir_json)), tmpdir, neff_name)

    compile_with_split._wait_split = True
    bu.compile_bir_kernel = compile_with_split
    if getattr(b2j, "compile_bir_kernel", None) is orig:
        b2j.compile_bir_kernel = compile_with_split


_program = None


def _build_program():
    """Build the single-core Bass/Tile program (same program on all 8 cores)."""
    import concourse.bass as bass
    import concourse.mybir as mybir
    from concourse.tile import TileContext

    f32 = mybir.dt.float32
    bf16 = mybir.dt.bfloat16
    fp8e3 = mybir.dt.float8e3
    Silu = mybir.ActivationFunctionType.Silu

    nc = bass.Bass()
    xb = nc.declare_dram_parameter("xb", [128, D], bf16, isOutput=False)
    ident = nc.declare_dram_parameter("ident", [128, 128], bf16, isOutput=False)
    w13a = nc.declare_dram_parameter(
        "w13a", [NSL - 1, NKD // 8, 128, 16 * SLW], fp8e3, isOutput=False
    )
    w13b = nc.declare_dram_parameter(
        "w13b", [NKD // 8, 128, 16 * SLW_LAST], fp8e3, isOutput=False
    )
    w2q8 = nc.declare_dram_parameter(
        "w2q8", [NQ, K8 // 2, 128, 2 * QW], fp8e3, isOutput=False
    )
    w2qb4 = nc.declare_dram_parameter(
        "w2qb4", [NQ, (KB16 - 1) // 2, 128, 2 * QW], bf16, isOutput=False
    )
    w2ql = nc.declare_dram_parameter("w2ql", [NQ, 128, QW], bf16, isOutput=False)
    oscale = nc.declare_dram_parameter("oscale", [128, 1], f32, isOutput=False)
    out = nc.declare_dram_parameter("out", [T, D], bf16, isOutput=True)

    def slw_of(s):
        return SLW_LAST if s == NSL - 1 else SLW

    def jw_of(ka):
        return JW_LAST if ka == NKA - 1 else 128

    with TileContext(nc) as tc:
        with (
            tc.tile_pool(name="singles", bufs=1) as singles,
            tc.tile_pool(name="w13p", bufs=4) as w13p,
            tc.tile_pool(name="w2p", bufs=24) as w2p,
            tc.tile_pool(name="gap", bufs=2) as gap,
            tc.tile_pool(name="hp", bufs=2) as hp,
            tc.tile_pool(name="outp", bufs=2) as outp,
            tc.tile_pool(name="psum_ug", bufs=2, space="PSUM") as psum_ug,
            tc.tile_pool(name="psum_t", bufs=2, space="PSUM") as psum_t,
            tc.tile_pool(name="psum_o", bufs=1, space="PSUM") as psum_o,
        ):
            # x first on the SP ring: gets hoisted to the preamble head
            # together with the first w13 tiles
            xb_s = singles.tile([128, D], bf16)
            nc.sync.dma_start(out=xb_s, in_=xb[:, :])
            osc_s = singles.tile([128, 1], f32)
            nc.scalar.dma_start(out=osc_s, in_=oscale[:, :])
            id_s = singles.tile([128, 128], bf16)
            nc.scalar.dma_start(out=id_s, in_=ident[:, :])
            hT = singles.tile([128, APAD], bf16)

            # pass 1: gate/up in [t, a] layout, x stationary, fp8 w13 moving.
            # w13 arrives in 4-kd batches (512 KB, 4 KB partition lines).
            for s in range(NSL):
                sw = slw_of(s)
                gate_ps = psum_ug.tile([128, sw], f32, name=f"g{s}", tag="gate")
                up_ps = psum_ug.tile([128, sw], f32, name=f"u{s}", tag="up")
                for g in range(NKD // 8):
                    w13t = w13p.tile([128, 16 * sw], fp8e3)
                    if s < NSL - 1:
                        nc.sync.dma_start(out=w13t, in_=w13a[s, g, :, :])
                    else:
                        nc.sync.dma_start(out=w13t, in_=w13b[g, :, :])
                    for m in range(8):
                        kd = 8 * g + m
                        lhsT = xb_s[:, kd * 128 : (kd + 1) * 128]
                        nc.tensor.matmul(
                            gate_ps,
                            lhsT,
                            w13t[:, m * 2 * sw : m * 2 * sw + sw],
                            start=(kd == 0),
                            stop=(kd == NKD - 1),
                        )
                        nc.tensor.matmul(
                            up_ps,
                            lhsT,
                            w13t[:, m * 2 * sw + sw : (m + 1) * 2 * sw],
                            start=(kd == 0),
                            stop=(kd == NKD - 1),
                        )
                ga = gap.tile([128, sw], f32)
                nc.scalar.activation(
                    out=ga, in_=gate_ps, func=Silu, scale=1.0 / WSCALE
                )
                hsl = hp.tile([128, sw], bf16)
                nc.vector.tensor_mul(out=hsl, in0=ga, in1=up_ps)
                # hT chunks via PE transpose + DVE copy. NOT the DMA XBAR:
                # a 128x128 XBAR transpose costs ~1.25us of DMA-engine
                # time (26 GB/s), and the single shared DMA engine is the
                # kernel's scarcest resource; 375ns on the PE is cheaper.
                for c in range(sw // 128):
                    ka = s * 4 + c
                    tp = psum_t.tile([128, 128], bf16, name=f"t{ka}", tag="tp")
                    nc.tensor.transpose(tp, hsl[:, c * 128 : (c + 1) * 128], id_s)
                    nc.vector.tensor_copy(
                        out=hT[:, ka * 128 : (ka + 1) * 128], in_=tp
                    )

            # pass 2: down projection in 4 d-quarters; each quarter
            # accumulates across all ka in 2 PSUM banks, then its output
            # de-scale copy + DMA overlaps the next quarter's compute.
            # ka < K8 chunks are e3m4 (2-ka pairs per 256 KB transfer),
            # the rest bf16; all pre-scaled by S2, undone by oscale.
            for q in range(NQ):
                ops = psum_o.tile([128, QW], f32, name=f"o{q}", tag="od")
                for kp in range(K8 // 2):
                    w2t = w2p.tile([128, 2 * QW], fp8e3)
                    nc.scalar.dma_start(out=w2t, in_=w2q8[q, kp, :, :])
                    for hf in range(2):
                        ka = 2 * kp + hf
                        lhsT = hT[:, ka * 128 : (ka + 1) * 128]
                        for b in range(2):
                            nc.tensor.matmul(
                                ops[:, b * 512 : (b + 1) * 512],
                                lhsT,
                                w2t[:, hf * QW + b * 512 : hf * QW + (b + 1) * 512],
                                start=(ka == 0),
                                stop=False,
                            )
                for r in range((KB16 - 1) // 2):
                    w2t = w2p.tile([128, 2 * QW], bf16)
                    nc.scalar.dma_start(out=w2t, in_=w2qb4[q, r, :, :])
                    for hf in range(2):
                        ka = K8 + 2 * r + hf
                        lhsT = hT[:, ka * 128 : (ka + 1) * 128]
                        for b in range(2):
                            nc.tensor.matmul(
                                ops[:, b * 512 : (b + 1) * 512],
                                lhsT,
                                w2t[:, hf * QW + b * 512 : hf * QW + (b + 1) * 512],
                                start=False,
                                stop=False,
                            )
                w2t = w2p.tile([128, QW], bf16)
                nc.scalar.dma_start(out=w2t[:JW_LAST], in_=w2ql[q, :JW_LAST, :])
                lhsT = hT[:JW_LAST, (NKA - 1) * 128 : NKA * 128]
                for b in range(2):
                    nc.tensor.matmul(
                        ops[:, b * 512 : (b + 1) * 512],
                        lhsT,
                        w2t[:JW_LAST, b * 512 : (b + 1) * 512],
                        start=False,
                        stop=True,
                    )
                oth = outp.tile([T, QW], bf16, name=f"oth{q}", tag="oth")
                nc.vector.tensor_scalar_mul(out=oth, in0=ops, scalar1=osc_s)
                # output store on the SP ring (idle once w13 is done):
                # doesn't queue behind the remaining w2 transfers on ACT
                nc.sync.dma_start(out=out[:, q * QW : (q + 1) * QW], in_=oth)

    return nc


def _pack_w13(w1s: np.ndarray, w3s: np.ndarray):
    """[ASH, D] f32 pair -> ([NSL-1, 4, 128, 16*SLW], [4, 128, 16*SLW_LAST])
    e3m4 blobs, scaled by WSCALE, batched 8 d-chunks per 1 MiB transfer
    (8 KB partition lines, the shape that sustains full DMA throughput).

    blob[s, g, p, m*2*sw + w*sw + j] = w{1,3}s[s*SLW + j, (8g+m)*128 + p].
    """
    padded = np.zeros((2, NSL * SLW, D), dtype=E3M4)
    padded[0, :ASH] = (w1s * np.float32(WSCALE)).astype(E3M4)
    padded[1, :ASH] = (w3s * np.float32(WSCALE)).astype(E3M4)
    # [2, NSL, SLW, NKD, 128] -> [s, kd, p, w, j]
    r = padded.reshape(2, NSL, SLW, NKD, 128).transpose(1, 3, 4, 0, 2)
    # [s, g, m, p, w, j] -> [s, g, p, m, w, j]
    r = r.reshape(NSL, NKD // 8, 8, 128, 2, SLW).transpose(0, 1, 3, 2, 4, 5)
    a = np.ascontiguousarray(r[: NSL - 1]).reshape(NSL - 1, NKD // 8, 128, 16 * SLW)
    b = np.ascontiguousarray(r[NSL - 1, :, :, :, :, :SLW_LAST]).reshape(
        NKD // 8, 128, 16 * SLW_LAST
    )
    return a, b


def _pack_w2(w2s: np.ndarray, scale: float):
    """[ASH, D] f32 -> (e3m4 pairs, bf16 pairs, last bf16 chunk) blobs.

    scale (= ew * S2 / WSCALE) is folded in; rows < K8*128 go e3m4 in
    2-ka pairs, rows up to (NKA-1)*128 bf16 in 2-ka pairs, the last
    (jw=51) chunk on its own so its DMA can be row-trimmed.
    """
    v = w2s * np.float32(scale)
    n8 = K8 * 128
    e = v[:n8].astype(E3M4)
    a = np.ascontiguousarray(
        e.reshape(K8 // 2, 2, 128, NQ, QW).transpose(3, 0, 2, 1, 4)
    ).reshape(NQ, K8 // 2, 128, 2 * QW)
    npair = (KB16 - 1) // 2
    bm = v[n8 : n8 + npair * 2 * 128].astype(BF16)
    b = np.ascontiguousarray(
        bm.reshape(npair, 2, 128, NQ, QW).transpose(3, 0, 2, 1, 4)
    ).reshape(NQ, npair, 128, 2 * QW)
    last = np.zeros((128, D), dtype=BF16)
    last[:JW_LAST] = v[(NKA - 1) * 128 :].astype(BF16)
    l = np.ascontiguousarray(last.reshape(128, NQ, QW).transpose(1, 0, 2))
    return a, b, l


def _pack_x(x: np.ndarray) -> np.ndarray:
    """[T, D] f32 -> [128, D] bf16: xb[p, kd*128 + t] = x[t, kd*128 + p]."""
    return (
        x.astype(BF16).reshape(T, NKD, 128).transpose(2, 1, 0).reshape(128, NKD * T)
    )


def make_in_maps(
    hidden_states,
    expert_weights,
    expert_ids,
    w1_e0,
    w3_e0,
    w2_e0,
    w1_e1,
    w3_e1,
    w2_e1,
):
    ids = np.asarray(expert_ids).reshape(-1)
    ew = np.asarray(expert_weights, dtype=np.float32).reshape(-1)
    if int(ids[0]) != 0:
        ew = ew[::-1]

    xb = _pack_x(np.asarray(hidden_states, dtype=np.float32))
    ident = np.eye(128, dtype=BF16)
    w1 = (np.asarray(w1_e0, np.float32), np.asarray(w1_e1, np.float32))
    w3 = (np.asarray(w3_e0, np.float32), np.asarray(w3_e1, np.float32))
    w2 = (np.asarray(w2_e0, np.float32), np.asarray(w2_e1, np.float32))

    in_maps = []
    for core in range(NCORES):
        e, r = divmod(core, 4)
        rows = slice(r * ASH, (r + 1) * ASH)
        w13a, w13b = _pack_w13(w1[e][rows], w3[e][rows])
        # h is scaled by WSCALE (up de-scale folded here), so w2 gets
        # ew / WSCALE, plus S2 to center the e3m4 chunks in range
        # (sigma_eff ~ 2); S2 is undone by oscale on the output copy.
        ewe = float(ew[e])
        s2 = 2.0 ** np.round(np.log2(6400.0 / max(abs(ewe), 1e-6)))
        w2q8, w2qb4, w2ql = _pack_w2(w2[e][rows], ewe * s2 / WSCALE)
        in_maps.append(
            {
                "xb": xb,
                "ident": ident,
                "w13a": w13a,
                "w13b": w13b,
                "w2q8": w2q8,
                "w2qb4": w2qb4,
                "w2ql": w2ql,
                "oscale": np.full((128, 1), 1.0 / s2, dtype=np.float32),
            }
        )
    return in_maps


LAST_RESULT = None


def kernel(**inputs) -> np.ndarray:
    global _program, LAST_RESULT
    _install_wait_split()
    from concourse.bass_utils import run_bass_kernel_spmd

    if _program is None:
        _program = _build_program()
        # Apply the BIR transforms at serialization time so the embedded
        # ant_bir payload (the compile-cache key) reflects them. Both
        # transforms are idempotent, so compile_bir_kernel re-applying
        # them is harmless.
        orig_tjb = _program.to_json_bytes

        def _tjb():
            return _split_multi_waits(_hoist_head_dmas(orig_tjb()))

        _program.to_json_bytes = _tjb

    in_maps = make_in_maps(**inputs)
    res = run_bass_kernel_spmd(
        _program,
        in_maps,
        core_ids=list(range(NCORES)),
        trace=bool(int(os.environ.get("KERNEL_TRACE", "0"))),
    )
    LAST_RESULT = res
    out = np.zeros((T, D), dtype=np.float32)
    for r in res.results:
        out += np.asarray(r["out"]).astype(np.float32)
    return out


# revision 48
# speedup vs baseline: 1.4270x; 1.0317x over previous
"""Trainium2 Bass kernel for nn_CachedMLP (2-expert dense MoE MLP).

Computation (reference):
    ew = expert_weights, swapped if expert_ids[0] != 0
    for e in {0,1}:  down_e = (silu(x @ w1_e.T) * (x @ w3_e.T)) @ w2_e
    out = down_0 * ew[0] + down_1 * ew[1]

Sharding: expert-parallel x tensor-parallel. Core c handles expert c//4
and rows [r*2867, (r+1)*2867) of that expert's w1/w3/w2 (r = c%4),
zero-padded to 2944 = 23*128. ew[e] (and the fp8 weight scale) is
folded into w2 on the host, so the sum of the 8 per-core partial
outputs is the final result.

Precision: w1/w3 are stored as fp8-E3M4 (scaled x64 on host; the PE
upconverts exactly, and silu de-scales via activation's scale=1/64);
w2, x, h stay bf16. Measured end-to-end rel err ~1.6% (tolerance 2e-2).
This halves the w13 HBM traffic: per-core reads drop 71.5 -> ~48 MB,
and the kernel is HBM-wire bound.

Device kernel per core:
  pass 1 over 6 a-slices (512 wide, last 384): for each d-chunk kd,
      gate[t, a-slice] += xT(kd) .T-matmul w1_tile(kd)   (PSUM, N=512)
      up[t, a-slice]   += xT(kd) .T-matmul w3_tile(kd)
    (x is the stationary operand -- loaded once per kd, amortized over
    both matmuls; the moving operand is the fp8 weight tile at N=512,
    so the PE streams near peak instead of reloading weights per MM).
    Then silu(scale=1/64)*up -> h[t, a-slice] (bf16), and PE-transpose
    128-col chunks into hT[a, t] for pass 2.
  pass 2: down[t, d-quarter] += hT(ka).T-matmul w2_tile(ka, q),
    accumulated over all 23 ka in 2 PSUM banks per 1024-wide quarter,
    then copied to SBUF and DMA'd out as bf16 partials.

DMA rings: w13 + hT transposes + out on SP (sync), w2 + x on ACT
(scalar) so w2 tiles prefetch during pass 1 without head-of-line
blocking behind w13.
"""

import json
import os

import ml_dtypes
import numpy as np

T = 128          # tokens
D = 4096         # hidden dim
ACTIVE = 11468   # sparsity-selected neurons per expert
NCORES = 8
ASH = ACTIVE // 4        # 2867 active rows per core
NKA = 23                 # 128-row a-chunks per core
APAD = NKA * 128         # 2944
NKD = D // 128           # 32 d-chunks
JW_LAST = ASH - (NKA - 1) * 128  # 51 useful rows in the last a-chunk
NSL = 6                  # a-slices in pass 1
SLW = 512                # slice width (last slice: 384)
SLW_LAST = APAD - 5 * SLW  # 384
NQ = 4                   # pass-2 d-quarters
QW = D // NQ             # 1024

WSCALE = 64.0            # fp8 weight scale for w1/w3
K8 = 14                  # leading 128-row a-chunks of w2 stored e3m4
KB16 = NKA - K8          # trailing chunks kept bf16 (8 paired + last jw=51)

BF16 = ml_dtypes.bfloat16
E3M4 = ml_dtypes.float8_e3m4

_EVENTSEM_CAP = 2


def _split_multi_waits(bir_json: bytes) -> bytes:
    """Hoist excess per-instruction sync waits into standalone waits.

    The axon-path walrus build accepts at most 1 sync-wait command per
    instruction (2 for EventSemaphore); Tile's wait assigner can emit
    more. Extra waits become wait-only EventSemaphore instructions
    inserted just before the offender on the same engine stream, which
    preserves semantics (the engine would have blocked there anyway).
    """
    d = json.loads(bir_json)
    for func in d.get("functions", []):
        for blk in func.get("blocks", []):
            out = []
            for inst in blk.get("instructions", []):
                sync = inst.get("sync_info")
                waits = (sync or {}).get("on_wait") or []
                cap = _EVENTSEM_CAP if inst.get("opcode") == "EventSemaphore" else 1
                if len(waits) > cap:
                    extra, keep = waits[:-cap], waits[-cap:]
                    for j in range(0, len(extra), _EVENTSEM_CAP):
                        w_inst = {
                            "engine": inst["engine"],
                            "ins": [],
                            "name": f"{inst['name']}-hw{j}",
                            "opcode": "EventSemaphore",
                            "outs": [],
                            "sync_info": {
                                "on_update": [],
                                "on_wait": extra[j : j + _EVENTSEM_CAP],
                            },
                        }
                        if "debug" in inst:
                            w_inst["debug"] = inst["debug"]
                        out.append(w_inst)
                    sync["on_wait"] = keep
                out.append(inst)
            blk["instructions"] = out
    return json.dumps(d).encode()


def _hoist_head_dmas(bir_json: bytes, max_hoist: int = 4) -> bytes:
    """Move the leading wait-free SP DMACopies to the head of the preamble.

    Tile's prologue (engine reg-init + const memsets + all-engine
    barrier rounds) takes ~8.5us before the first dma_start issues,
    leaving the HBM wire idle. The first input DMAs have no waits and
    their destination tiles are disjoint from everything the preamble
    writes (asserted below), so issuing them at the very front of the
    SP stream is safe: DGE rings and DMA semaphores are initialized by
    the runtime at NEFF load (not by the preamble), HWDGE keeps
    per-engine FIFO order, and the sem increments are only ever waited
    on with >= thresholds.
    """
    d = json.loads(bir_json)
    for func in d.get("functions", []):
        blocks = func.get("blocks", [])
        if len(blocks) < 2:
            continue
        main, tile_blk = blocks[0], blocks[1]
        if main.get("name") != "main" or not tile_blk.get("name", "").startswith(
            "tile_context"
        ):
            continue
        # preamble must write only const tiles, else hoisting is unsafe
        pre_outs = {
            o.get("memref")
            for inst in main["instructions"]
            for o in inst.get("outs", [])
            if isinstance(o, dict)
        }
        if any(m and not m.startswith("const-") for m in pre_outs):
            continue
        hoisted = []
        remaining = []
        for inst in tile_blk["instructions"]:
            if (
                len(hoisted) < max_hoist
                and inst.get("engine") == "SP"
                and inst.get("opcode") == "DMACopy"
                and not ((inst.get("sync_info") or {}).get("on_wait"))
            ):
                hoisted.append(inst)
            else:
                remaining.append(inst)
        if not hoisted:
            continue
        main["instructions"][0:0] = hoisted
        tile_blk["instructions"] = remaining
    return json.dumps(d).encode()


def _install_wait_split():
    import concourse.bass2jax as b2j
    import concourse.bass_utils as bu

    if getattr(bu.compile_bir_kernel, "_wait_split", False):
        return
    orig = bu.compile_bir_kernel

    def compile_with_split(bir_json, tmpdir, neff_name="file.neff"):
        return orig(_split_multi_waits(_hoist_head_dmas(bir_json)), tmpdir, neff_name)

    compile_with_split._wait_split = True
    bu.compile_bir_kernel = compile_with_split
    if getattr(b2j, "compile_bir_kernel", None) is orig:
        b2j.compile_bir_kernel = compile_with_split


_program = None


def _build_program():
    """Build the single-core Bass/Tile program (same program on all 8 cores)."""
    import concourse.bass as bass
    import concourse.mybir as mybir
    from concourse.tile import TileContext

    f32 = mybir.dt.float32
    bf16 = mybir.dt.bfloat16
    fp8e3 = mybir.dt.float8e3
    Silu = mybir.ActivationFunctionType.Silu

    nc = bass.Bass()
    xb = nc.declare_dram_parameter("xb", [128, D], bf16, isOutput=False)
    ident = nc.declare_dram_parameter("ident", [128, 128], bf16, isOutput=False)
    w13a = nc.declare_dram_parameter(
        "w13a", [NSL - 1, NKD // 8, 128, 16 * SLW], fp8e3, isOutput=False
    )
    w13b = nc.declare_dram_parameter(
        "w13b", [NKD // 8, 128, 16 * SLW_LAST], fp8e3, isOutput=False
    )
    w2q8 = nc.declare_dram_parameter(
        "w2q8", [NQ, K8 // 2, 128, 2 * QW], fp8e3, isOutput=False
    )
    w2qb4 = nc.declare_dram_parameter(
        "w2qb4", [NQ, (KB16 - 1) // 2, 128, 2 * QW], bf16, isOutput=False
    )
    w2ql = nc.declare_dram_parameter("w2ql", [NQ, 128, QW], bf16, isOutput=False)
    oscale = nc.declare_dram_parameter("oscale", [128, 1], f32, isOutput=False)
    out = nc.declare_dram_parameter("out", [T, D], bf16, isOutput=True)

    def slw_of(s):
        return SLW_LAST if s == NSL - 1 else SLW

    def jw_of(ka):
        return JW_LAST if ka == NKA - 1 else 128

    with TileContext(nc) as tc:
        with (
            tc.tile_pool(name="singles", bufs=1) as singles,
            tc.tile_pool(name="w13p", bufs=4) as w13p,
            tc.tile_pool(name="w2p", bufs=16) as w2p,
            tc.tile_pool(name="gap", bufs=2) as gap,
            tc.tile_pool(name="hp", bufs=2) as hp,
            tc.tile_pool(name="outp", bufs=2) as outp,
            tc.tile_pool(name="psum_ug", bufs=2, space="PSUM") as psum_ug,
            tc.tile_pool(name="psum_t", bufs=2, space="PSUM") as psum_t,
            tc.tile_pool(name="psum_o", bufs=1, space="PSUM") as psum_o,
        ):
            # x on the SP ring, split into quarters: d-chunk group g only
            # needs quarter g, so quarter 0 goes first and quarters 1-3
            # are interleaved ahead of their w13 group in slice 0 below.
            # This gets the first matmul going after ~1.3 MB of wire.
            xb_s = singles.tile([128, D], bf16)

            def xb_quarter(xq):
                nc.sync.dma_start(
                    out=xb_s[:, xq * (D // 4) : (xq + 1) * (D // 4)],
                    in_=xb[:, xq * (D // 4) : (xq + 1) * (D // 4)],
                )

            xb_quarter(0)
            osc_s = singles.tile([128, 1], f32)
            nc.scalar.dma_start(out=osc_s, in_=oscale[:, :])
            id_s = singles.tile([128, 128], bf16)
            nc.scalar.dma_start(out=id_s, in_=ident[:, :])
            hT = singles.tile([128, APAD], bf16)

            # pass-2 plumbing, shared between the interleaved quarter 0
            # (emitted inside pass 1 as its hT chunks appear) and the
            # trailing quarters 1-3.
            def mm_pair(ops, w2t, ka0):
                for hf in range(2):
                    ka = ka0 + hf
                    lhsT = hT[:, ka * 128 : (ka + 1) * 128]
                    for b in range(2):
                        nc.tensor.matmul(
                            ops[:, b * 512 : (b + 1) * 512],
                            lhsT,
                            w2t[:, hf * QW + b * 512 : hf * QW + (b + 1) * 512],
                            start=(ka == 0),
                            stop=False,
                            skip_group_check=True,
                        )

            def emit_pairs(q, ops, s):
                # ka pairs whose hT chunks slice s produced (ka = 4s..4s+3)
                for half in range(2):
                    ka0 = 4 * s + 2 * half
                    if ka0 + 1 < K8:
                        w2t = w2p.tile([128, 2 * QW], fp8e3)
                        nc.scalar.dma_start(out=w2t, in_=w2q8[q, ka0 // 2, :, :])
                        mm_pair(ops, w2t, ka0)
                    elif ka0 + 1 < NKA:
                        w2t = w2p.tile([128, 2 * QW], bf16)
                        nc.scalar.dma_start(
                            out=w2t, in_=w2qb4[q, (ka0 - K8) // 2, :, :]
                        )
                        mm_pair(ops, w2t, ka0)

            def emit_last(q, ops):
                w2t = w2p.tile([128, QW], bf16)
                nc.scalar.dma_start(out=w2t[:JW_LAST], in_=w2ql[q, :JW_LAST, :])
                lhsT = hT[:JW_LAST, (NKA - 1) * 128 : NKA * 128]
                for b in range(2):
                    nc.tensor.matmul(
                        ops[:, b * 512 : (b + 1) * 512],
                        lhsT,
                        w2t[:JW_LAST, b * 512 : (b + 1) * 512],
                        start=False,
                        stop=True,
                        skip_group_check=True,
                    )

            def emit_store(q, ops):
                oth = outp.tile([T, QW], bf16, name=f"oth{q}", tag="oth")
                nc.vector.tensor_scalar_mul(out=oth, in0=ops, scalar1=osc_s)
                # output store on the SP ring (idle once w13 is done):
                # doesn't queue behind the remaining w2 transfers on ACT
                nc.sync.dma_start(out=out[:, q * QW : (q + 1) * QW], in_=oth)

            # pass 1: gate/up in [t, a] layout, x stationary, fp8 w13 moving.
            # w13 arrives in 8-kd batches (1 MB, 8 KB partition lines).
            # Quarter 0 of the down projection rides along: its ka-pair
            # matmuls are emitted right after the slice that produced
            # their hT chunks, filling pass 1's DMA-wait gaps on the PE.
            ops_q0 = psum_o.tile([128, QW], f32, name="o0", tag="od")
            for s in range(NSL):
                sw = slw_of(s)
                gate_ps = psum_ug.tile([128, sw], f32, name=f"g{s}", tag="gate")
                up_ps = psum_ug.tile([128, sw], f32, name=f"u{s}", tag="up")
                for g in range(NKD // 8):
                    if s == 0 and g >= 1:
                        xb_quarter(g)
                    w13t = w13p.tile([128, 16 * sw], fp8e3)
                    if s == 0 and g == 0:
                        # split the very first tile so the first matmuls
                        # start after 512 KB instead of 1 MB
                        hw_ = 8 * sw
                        nc.sync.dma_start(
                            out=w13t[:, :hw_], in_=w13a[0, 0, :, :hw_]
                        )
                        nc.sync.dma_start(
                            out=w13t[:, hw_:], in_=w13a[0, 0, :, hw_:]
                        )
                    elif s < NSL - 1:
                        nc.sync.dma_start(out=w13t, in_=w13a[s, g, :, :])
                    else:
                        nc.sync.dma_start(out=w13t, in_=w13b[g, :, :])
                    for m in range(8):
                        kd = 8 * g + m
                        lhsT = xb_s[:, kd * 128 : (kd + 1) * 128]
                        nc.tensor.matmul(
                            gate_ps,
                            lhsT,
                            w13t[:, m * 2 * sw : m * 2 * sw + sw],
                            start=(kd == 0),
                            stop=(kd == NKD - 1),
                        )
                        nc.tensor.matmul(
                            up_ps,
                            lhsT,
                            w13t[:, m * 2 * sw + sw : (m + 1) * 2 * sw],
                            start=(kd == 0),
                            stop=(kd == NKD - 1),
                        )
                ga = gap.tile([128, sw], f32)
                nc.scalar.activation(
                    out=ga, in_=gate_ps, func=Silu, scale=1.0 / WSCALE
                )
                hsl = hp.tile([128, sw], bf16)
                nc.vector.tensor_mul(out=hsl, in0=ga, in1=up_ps)
                # hT chunks via PE transpose + DVE copy. NOT the DMA XBAR:
                # a 128x128 XBAR transpose costs ~1.25us of DMA-engine
                # time (26 GB/s), and the single shared DMA engine is the
                # kernel's scarcest resource; 375ns on the PE is cheaper.
                for c in range(sw // 128):
                    ka = s * 4 + c
                    tp = psum_t.tile([128, 128], bf16, name=f"t{ka}", tag="tp")
                    nc.tensor.transpose(tp, hsl[:, c * 128 : (c + 1) * 128], id_s)
                    nc.vector.tensor_copy(
                        out=hT[:, ka * 128 : (ka + 1) * 128], in_=tp
                    )
                emit_pairs(0, ops_q0, s)
            emit_last(0, ops_q0)
            emit_store(0, ops_q0)

            # pass 2 remainder: quarters 1-3, each accumulating across all
            # ka in 2 PSUM banks (sequential via the bufs=1 pool), output
            # de-scale copy + DMA overlapping the next quarter's compute.
            for q in range(1, NQ):
                ops = psum_o.tile([128, QW], f32, name=f"o{q}", tag="od")
                for s in range(NSL):
                    emit_pairs(q, ops, s)
                emit_last(q, ops)
                emit_store(q, ops)

    return nc


def _pack_w13(w1s: np.ndarray, w3s: np.ndarray):
    """[ASH, D] f32 pair -> ([NSL-1, 4, 128, 16*SLW], [4, 128, 16*SLW_LAST])
    e3m4 blobs, scaled by WSCALE, batched 8 d-chunks per 1 MiB transfer
    (8 KB partition lines, the shape that sustains full DMA throughput).

    blob[s, g, p, m*2*sw + w*sw + j] = w{1,3}s[s*SLW + j, (8g+m)*128 + p].
    """
    padded = np.zeros((2, NSL * SLW, D), dtype=E3M4)
    padded[0, :ASH] = (w1s * np.float32(WSCALE)).astype(E3M4)
    padded[1, :ASH] = (w3s * np.float32(WSCALE)).astype(E3M4)
    # [2, NSL, SLW, NKD, 128] -> [s, kd, p, w, j]
    r = padded.reshape(2, NSL, SLW, NKD, 128).transpose(1, 3, 4, 0, 2)
    # [s, g, m, p, w, j] -> [s, g, p, m, w, j]
    r = r.reshape(NSL, NKD // 8, 8, 128, 2, SLW).transpose(0, 1, 3, 2, 4, 5)
    a = np.ascontiguousarray(r[: NSL - 1]).reshape(NSL - 1, NKD // 8, 128, 16 * SLW)
    b = np.ascontiguousarray(r[NSL - 1, :, :, :, :, :SLW_LAST]).reshape(
        NKD // 8, 128, 16 * SLW_LAST
    )
    return a, b


def _pack_w2(w2s: np.ndarray, scale: float):
    """[ASH, D] f32 -> (e3m4 pairs, bf16 pairs, last bf16 chunk) blobs.

    scale (= ew * S2 / WSCALE) is folded in; rows < K8*128 go e3m4 in
    2-ka pairs, rows up to (NKA-1)*128 bf16 in 2-ka pairs, the last
    (jw=51) chunk on its own so its DMA can be row-trimmed.
    """
    v = w2s * np.float32(scale)
    n8 = K8 * 128
    e = v[:n8].astype(E3M4)
    a = np.ascontiguousarray(
        e.reshape(K8 // 2, 2, 128, NQ, QW).transpose(3, 0, 2, 1, 4)
    ).reshape(NQ, K8 // 2, 128, 2 * QW)
    npair = (KB16 - 1) // 2
    bm = v[n8 : n8 + npair * 2 * 128].astype(BF16)
    b = np.ascontiguousarray(
        bm.reshape(npair, 2, 128, NQ, QW).transpose(3, 0, 2, 1, 4)
    ).reshape(NQ, npair, 128, 2 * QW)
    last = np.zeros((128, D), dtype=BF16)
    last[:JW_LAST] = v[(NKA - 1) * 128 :].astype(BF16)
    l = np.ascontiguousarray(last.reshape(128, NQ, QW).transpose(1, 0, 2))
    return a, b, l


def _pack_x(x: np.ndarray) -> np.ndarray:
    """[T, D] f32 -> [128, D] bf16: xb[p, kd*128 + t] = x[t, kd*128 + p]."""
    return (
        x.astype(BF16).reshape(T, NKD, 128).transpose(2, 1, 0).reshape(128, NKD * T)
    )


def make_in_maps(
    hidden_states,
    expert_weights,
    expert_ids,
    w1_e0,
    w3_e0,
    w2_e0,
    w1_e1,
    w3_e1,
    w2_e1,
):
    ids = np.asarray(expert_ids).reshape(-1)
    ew = np.asarray(expert_weights, dtype=np.float32).reshape(-1)
    if int(ids[0]) != 0:
        ew = ew[::-1]

    xb = _pack_x(np.asarray(hidden_states, dtype=np.float32))
    ident = np.eye(128, dtype=BF16)
    w1 = (np.asarray(w1_e0, np.float32), np.asarray(w1_e1, np.float32))
    w3 = (np.asarray(w3_e0, np.float32), np.asarray(w3_e1, np.float32))
    w2 = (np.asarray(w2_e0, np.float32), np.asarray(w2_e1, np.float32))

    in_maps = []
    for core in range(NCORES):
        e, r = divmod(core, 4)
        rows = slice(r * ASH, (r + 1) * ASH)
        w13a, w13b = _pack_w13(w1[e][rows], w3[e][rows])
        # h is scaled by WSCALE (up de-scale folded here), so w2 gets
        # ew / WSCALE, plus S2 to center the e3m4 chunks in range
        # (sigma_eff ~ 2); S2 is undone by oscale on the output copy.
        ewe = float(ew[e])
        s2 = 2.0 ** np.round(np.log2(6400.0 / max(abs(ewe), 1e-6)))
        w2q8, w2qb4, w2ql = _pack_w2(w2[e][rows], ewe * s2 / WSCALE)
        in_maps.append(
            {
                "xb": xb,
                "ident": ident,
                "w13a": w13a,
                "w13b": w13b,
                "w2q8": w2q8,
                "w2qb4": w2qb4,
                "w2ql": w2ql,
                "oscale": np.full((128, 1), 1.0 / s2, dtype=np.float32),
            }
        )
    return in_maps


LAST_RESULT = None


def kernel(**inputs) -> np.ndarray:
    global _program, LAST_RESULT
    _install_wait_split()
    from concourse.bass_utils import run_bass_kernel_spmd

    if _program is None:
        _program = _build_program()
        # Apply the BIR transforms at serialization time so the embedded
        # ant_bir payload (the compile-cache key) reflects them. Both
        # transforms are idempotent, so compile_bir_kernel re-applying
        # them is harmless.
        orig_tjb = _program.to_json_bytes

        def _tjb():
            return _split_multi_waits(_hoist_head_dmas(orig_tjb()))

        _program.to_json_bytes = _tjb

    in_maps = make_in_maps(**inputs)
    res = run_bass_kernel_spmd(
        _program,
        in_maps,
        core_ids=list(range(NCORES)),
        trace=bool(int(os.environ.get("KERNEL_TRACE", "0"))),
    )
    LAST_RESULT = res
    out = np.zeros((T, D), dtype=np.float32)
    for r in res.results:
        out += np.asarray(r["out"]).astype(np.float32)
    return out
